# revision 9
# baseline (speedup 1.0000x reference)
"""Trainium2 Bass kernel for the Context Encoder problem:

    ce  = c2e_weight[nodes]            # [N, 128] embedding gather
    h   = relu(ce @ w1.T + b1)         # [N, 128]
    out = relu(h @ w2.T + b2)          # [N, 128]

Strategy (8 NeuronCores, vocab-range sharding):
  200000 node ids over a 100000-row vocab hit ~86.5% of all vocab rows,
  so transforming the table itself is less work than gathering per-node
  rows (and avoids the per-index DMA descriptor-generation cost that
  dominates any on-device gather).

  - The host computes the unique referenced vocab ids (~86.5k), packs
    their d-major (transposed) table columns into a dense [128, 11264]
    bf16 slice per core, and after the run scatters the transformed
    columns back (t2[ids] = slice.T) before the final t2[nodes] gather.
    Rows never referenced are never moved or computed.  A full-table
    (no-compaction) variant is kept as a lazily-compiled fallback for
    distributions that reference more rows than the packed capacity.
  - Everything on the wire is bf16 (table in, result out, both weight
    matrices): the 2e-2 rel-err budget dwarfs bf16 rounding (~4e-3
    measured end-to-end), DMA bytes halve vs f32, and the PE runs bf16
    at 1 cycle/row vs 4 for f32.  PSUM accumulation stays f32.
  - All input-window DMAs are issued up-front on the Sync queue into
    dedicated SBUF buffers (sizes ramp 4,8,16,24.. blocks so the first
    matmul starts within ~0.5us) — the in-order queue then never stalls
    an input transfer behind an output DMA whose data isn't ready, and
    the DMA subsystem streams at full bandwidth from t=0.  Weight/bias
    loads issue from the Activation queue so they don't delay the
    window stream.
  - d-major input feeds mm1 directly (lhsT = w1.T stationary, window as
    the moving operand); mm2 keeps w2.T stationary.  Supertiles are
    processed in pairs sharing a 2-bank [128, 1024] PSUM tile, so each
    relu+bias (fused into one ScalarE activation or one VectorE dual-op
    tensor_scalar, alternating between ACT and DVE per pair for engine
    balance) and each output DMA (2 KB per partition line) covers two
    supertiles.  No PE transposes, no PSUM staging copies, no bias
    matmuls.
  - Results stay feature-major; the host transposes each window and
    maps node positions to rows (out = T2[nodes]) as the unshard step.
"""

import os
import sys

for _p in ("/opt/trn_rl_repo",):
    if _p not in sys.path:
        sys.path.insert(0, _p)

import numpy as np

import concourse.bass as bass
import concourse.mybir as mybir
import concourse.tile as tile
from concourse import bacc
from concourse.bass_utils import run_bass_kernel_spmd
from concourse.tile import TileContext

P = 128
D = 128
N_CORES = 8
VOCAB = 100000
CAP_BLOCKS = 88            # compacted rows per core (88*128 = 11264)
FULL_BLOCKS = 100          # full-table fallback (12800 >= 12500 owned)
RANGE = VOCAB // N_CORES   # 12500 vocab rows owned per core (fallback)
G = 4                      # blocks per compute super-tile (free dim 512)


def _make_chunks(blocks: int) -> list[int]:
    """Input-DMA chunk sizes in blocks: ramp up so compute starts early,
    then large chunks for efficient streaming.  All chunks are multiples
    of 8 blocks (a compute pair) except a possible 4-block tail, so no
    pair ever waits on two input DMAs."""
    chunks = [8, 16]
    while sum(chunks) < blocks:
        chunks.append(min(24, blocks - sum(chunks)))
    assert sum(chunks) == blocks and all(c % G == 0 for c in chunks)
    assert all(c % (2 * G) == 0 for c in chunks[:-1])
    return chunks


def build_nc(blocks: int):
    f32 = mybir.dt.float32
    bf16 = mybir.dt.bfloat16
    nc = bacc.Bacc("TRN2", target_bir_lowering=False, debug=False,
                   num_devices=N_CORES, enable_partition_id=False)

    rows = blocks * P
    tsl_t = nc.dram_tensor("tslice", [P, rows], bf16,
                           kind="ExternalInput").ap()
    w1t_t = nc.dram_tensor("w1t", [D, D], bf16, kind="ExternalInput").ap()
    w2t_t = nc.dram_tensor("w2t", [D, D], bf16, kind="ExternalInput").ap()
    b1_t = nc.dram_tensor("b1c", [P, 1], f32, kind="ExternalInput").ap()
    b2_t = nc.dram_tensor("b2c", [P, 1], f32, kind="ExternalInput").ap()
    out_t = nc.dram_tensor("out", [P, rows], bf16,
                           kind="ExternalOutput").ap()

    fw = G * D          # super-tile free width (512)
    pw = 2 * fw         # pair width (1024)
    chunks = _make_chunks(blocks)

    with TileContext(nc) as tc:
        with (
            tc.tile_pool(name="const", bufs=1) as cpool,
            tc.tile_pool(name="win", bufs=1) as gpool,
            tc.tile_pool(name="hsb", bufs=8) as hsbpool,
            tc.tile_pool(name="osb", bufs=3) as osbpool,
            tc.tile_pool(name="hps", bufs=4, space="PSUM") as hpool,
            tc.tile_pool(name="ops", bufs=2, space="PSUM") as opool,
        ):  # PSUM: h 4x1 bank + o 2x2 banks = all 8 banks
            # weight/bias loads on the ACT queue: tiny transfers that must
            # not delay the window stream on the Sync queue
            w1t_sb = cpool.tile([D, D], bf16, tag="w1t")
            nc.scalar.dma_start(out=w1t_sb[:], in_=w1t_t[:])
            w2t_sb = cpool.tile([D, D], bf16, tag="w2t")
            nc.scalar.dma_start(out=w2t_sb[:], in_=w2t_t[:])
            b1_sb = cpool.tile([P, 1], f32, tag="b1")
            nc.scalar.dma_start(out=b1_sb[:], in_=b1_t[:])
            b2_sb = cpool.tile([P, 1], f32, tag="b2")
            nc.scalar.dma_start(out=b2_sb[:], in_=b2_t[:])

            # every window DMA up-front: in-order Sync queue streams the
            # whole input without ever waiting behind an output DMA
            sts = []  # supertile -> (window tile, column offset)
            r0 = 0
            for ci, cb in enumerate(chunks):
                win = gpool.tile([P, cb * D], bf16, tag=f"win{ci}")
                nc.sync.dma_start(out=win[:], in_=tsl_t[:, r0 : r0 + cb * P])
                for s in range(cb // G):
                    sts.append((win, s * fw))
                r0 += cb * P

            def relu_bias(out_ap, in_ap, bias_sb, on_act: bool):
                if on_act:
                    nc.scalar.activation(out_ap, in_ap,
                                         mybir.ActivationFunctionType.Relu,
                                         bias=bias_sb[:, 0:1])
                else:
                    nc.vector.tensor_scalar(
                        out=out_ap, in0=in_ap, scalar1=bias_sb[:, 0:1],
                        scalar2=0.0, op0=mybir.AluOpType.add,
                        op1=mybir.AluOpType.max)

            # layer 1 is emitted supertile-granular (mm1 -> ACT relu1 into
            # a bf16 SBUF staging tile), layer 2 pair-granular (2x mm2
            # into a 2-bank PSUM tile -> DVE relu2 -> output DMA).  mm1
            # emission runs ahead of mm2 with a look-ahead that grows from
            # 2 supertiles (so the first outputs aren't gated on later
            # input chunks) to 6 (so the in-order PE queue never waits a
            # full relu1 latency in steady state).
            n_super = len(sts)
            hT = [None] * n_super
            next1 = 0

            def emit_mm1():
                nonlocal next1
                s = next1
                wt, off = sts[s]
                h_ps = hpool.tile([P, fw], f32, tag="h")
                nc.tensor.matmul(out=h_ps[:], lhsT=w1t_sb[:],
                                 rhs=wt[:, off : off + fw],
                                 start=True, stop=True)
                hT[s] = hsbpool.tile([P, fw], bf16, tag="hT",
                                     name=f"hT{s}")
                relu_bias(hT[s][:], h_ps[:], b1_sb, on_act=True)
                next1 += 1

            n_pairs = (n_super + 1) // 2
            for k in range(n_pairs):
                s0 = 2 * k
                w = min(2, n_super - s0) * fw
                target = min(n_super, s0 + 2 + min(2 + 2 * k, 4))
                while next1 < target:
                    emit_mm1()

                o_ps = opool.tile([P, pw], f32, tag="o")
                for j in range(w // fw):
                    nc.tensor.matmul(out=o_ps[:, j * fw : (j + 1) * fw],
                                     lhsT=w2t_sb[:],
                                     rhs=hT[s0 + j][:],
                                     start=True, stop=True)
                    hT[s0 + j] = None
                o_sb = osbpool.tile([P, pw], bf16, tag="o_sb")
                relu_bias(o_sb[:, :w], o_ps[:, :w], b2_sb, on_act=False)

                nc.sync.dma_start(out=out_t[:, s0 * fw : s0 * fw + w],
                                  in_=o_sb[:, :w])

    nc.compile()
    return nc


_CACHED_NC: dict = {}
LAST_RESULTS = None


def _get_nc(blocks: int):
    if blocks not in _CACHED_NC:
        _CACHED_NC[blocks] = build_nc(blocks)
    return _CACHED_NC[blocks]


def _run(in_maps):
    trace = os.environ.get("BASS_KERNEL_TRACE") == "1"
    if trace:
        try:  # tracing needs the NTFF hook; degrade silently without it
            import antenv.axon_hooks  # noqa: F401
        except ImportError:
            trace = False
    nc = _get_nc(in_maps[0]["tslice"].shape[1] // P)
    res = run_bass_kernel_spmd(nc, in_maps, core_ids=list(range(N_CORES)),
                               trace=trace)
    global LAST_RESULTS
    LAST_RESULTS = res
    return res


def kernel(nodes, c2e_weight, w1, b1, w2, b2):
    import ml_dtypes

    bf16 = ml_dtypes.bfloat16
    nodes = np.asarray(nodes)
    c2e_weight = np.asarray(c2e_weight, dtype=np.float32)
    w1 = np.asarray(w1, dtype=np.float32)
    b1 = np.asarray(b1, dtype=np.float32)
    w2 = np.asarray(w2, dtype=np.float32)
    b2 = np.asarray(b2, dtype=np.float32)

    vocab = c2e_weight.shape[0]
    assert vocab == VOCAB, vocab

    tableT = np.ascontiguousarray(c2e_weight.T.astype(bf16))  # [128, VOCAB]
    w1t = np.ascontiguousarray(w1.T.astype(bf16))
    w2t = np.ascontiguousarray(w2.T.astype(bf16))
    b1c = np.ascontiguousarray(b1.reshape(P, 1))
    b2c = np.ascontiguousarray(b2.reshape(P, 1))
    consts = {"w1t": w1t, "w2t": w2t, "b1c": b1c, "b2c": b2c}

    uniq = np.unique(nodes)
    cap = N_CORES * CAP_BLOCKS * P
    t2 = np.empty((vocab, D), dtype=np.float32)

    if len(uniq) <= cap:
        # compacted path: move/compute only referenced rows (padded with
        # duplicates of id 0, which all receive the same correct value)
        rows = CAP_BLOCKS * P
        ids = np.zeros(cap, dtype=np.int64)
        ids[: len(uniq)] = uniq
        packed = tableT[:, ids]  # [128, cap] bf16
        in_maps = []
        for i in range(N_CORES):
            in_maps.append({
                "tslice": np.ascontiguousarray(
                    packed[:, i * rows : (i + 1) * rows]),
                **consts,
            })
        res = _run(in_maps)
        for i in range(N_CORES):
            dense = res.results[i]["out"]  # [128, rows] bf16, (d, r)
            t2[ids[i * rows : (i + 1) * rows]] = \
                dense.T.astype(np.float32)
    else:
        # fallback: transform the whole table in fixed vocab ranges
        rows = FULL_BLOCKS * P
        starts, in_maps = [], []
        for i in range(N_CORES):
            start = min(i * RANGE, vocab - rows)
            starts.append(start)
            in_maps.append({
                "tslice": np.ascontiguousarray(
                    tableT[:, start : start + rows]),
                **consts,
            })
        res = _run(in_maps)
        for i in range(N_CORES):
            dense = res.results[i]["out"]
            lo = i * RANGE
            hi = min((i + 1) * RANGE, vocab)
            t2[lo:hi] = dense[:, lo - starts[i] : hi - starts[i]].T \
                .astype(np.float32)

    return t2[nodes]


# revision 12
# speedup vs baseline: 1.0390x; 1.0390x over previous
"""Trainium2 Bass kernel for the Context Encoder problem:

    ce  = c2e_weight[nodes]            # [N, 128] embedding gather
    h   = relu(ce @ w1.T + b1)         # [N, 128]
    out = relu(h @ w2.T + b2)          # [N, 128]

Strategy (8 NeuronCores, vocab-range sharding):
  200000 node ids over a 100000-row vocab hit ~86.5% of all vocab rows,
  so transforming the table itself is less work than gathering per-node
  rows (and avoids the per-index DMA descriptor-generation cost that
  dominates any on-device gather).

  - The host computes the unique referenced vocab ids (~86.5k), packs
    their d-major (transposed) table columns into a dense [128, 11264]
    bf16 slice per core, and after the run scatters the transformed
    columns back (t2[ids] = slice.T) before the final t2[nodes] gather.
    Rows never referenced are never moved or computed.  A full-table
    (no-compaction) variant is kept as a lazily-compiled fallback for
    distributions that reference more rows than the packed capacity.
  - Everything on the wire is bf16 (table in, result out, both weight
    matrices): the 2e-2 rel-err budget dwarfs bf16 rounding (~4e-3
    measured end-to-end), DMA bytes halve vs f32, and the PE runs bf16
    at 1 cycle/row vs 4 for f32.  PSUM accumulation stays f32.
  - All input-window DMAs are issued up-front on the Sync queue into
    dedicated SBUF buffers (sizes ramp 4,8,16,24.. blocks so the first
    matmul starts within ~0.5us) — the in-order queue then never stalls
    an input transfer behind an output DMA whose data isn't ready, and
    the DMA subsystem streams at full bandwidth from t=0.  Weight/bias
    loads issue from the Activation queue so they don't delay the
    window stream.
  - d-major input feeds mm1 directly (lhsT = w1.T stationary, window as
    the moving operand); mm2 keeps w2.T stationary.  Supertiles are
    processed in pairs sharing a 2-bank [128, 1024] PSUM tile, so each
    relu+bias (fused into one ScalarE activation or one VectorE dual-op
    tensor_scalar, alternating between ACT and DVE per pair for engine
    balance) and each output DMA (2 KB per partition line) covers two
    supertiles.  No PE transposes, no PSUM staging copies, no bias
    matmuls.
  - Results stay feature-major; the host transposes each window and
    maps node positions to rows (out = T2[nodes]) as the unshard step.
"""

import os
import sys

for _p in ("/opt/trn_rl_repo",):
    if _p not in sys.path:
        sys.path.insert(0, _p)

import numpy as np

import concourse.bass as bass
import concourse.mybir as mybir
import concourse.tile as tile
from concourse import bacc
from concourse.bass_utils import run_bass_kernel_spmd
from concourse.tile import TileContext

P = 128
D = 128
N_CORES = 8
VOCAB = 100000
CAP_BLOCKS = 88            # compacted rows per core (88*128 = 11264)
FULL_BLOCKS = 100          # full-table fallback (12800 >= 12500 owned)
RANGE = VOCAB // N_CORES   # 12500 vocab rows owned per core (fallback)
G = 4                      # blocks per compute super-tile (free dim 512)


def _make_chunks(blocks: int) -> list[int]:
    """Input-DMA chunk sizes in blocks: ramp up so compute starts early,
    then large chunks for efficient streaming.  All chunks are multiples
    of 8 blocks (a compute pair) except a possible 4-block tail, so no
    pair ever waits on two input DMAs."""
    chunks = [8, 16]
    while sum(chunks) < blocks:
        chunks.append(min(24, blocks - sum(chunks)))
    assert sum(chunks) == blocks and all(c % G == 0 for c in chunks)
    assert all(c % (2 * G) == 0 for c in chunks[:-1])
    return chunks


def build_nc(blocks: int):
    f32 = mybir.dt.float32
    bf16 = mybir.dt.bfloat16
    nc = bacc.Bacc("TRN2", target_bir_lowering=False, debug=False,
                   num_devices=N_CORES, enable_partition_id=False)

    rows = blocks * P
    tsl_t = nc.dram_tensor("tslice", [P, rows], bf16,
                           kind="ExternalInput").ap()
    w1t_t = nc.dram_tensor("w1t", [D, D], bf16, kind="ExternalInput").ap()
    w2t_t = nc.dram_tensor("w2t", [D, D], bf16, kind="ExternalInput").ap()
    b1_t = nc.dram_tensor("b1c", [P, 1], f32, kind="ExternalInput").ap()
    b2_t = nc.dram_tensor("b2c", [P, 1], f32, kind="ExternalInput").ap()
    out_t = nc.dram_tensor("out", [P, rows], bf16,
                           kind="ExternalOutput").ap()

    fw = G * D          # super-tile free width (512)
    pw = 2 * fw         # pair width (1024)
    chunks = _make_chunks(blocks)

    with TileContext(nc) as tc:
        with (
            tc.tile_pool(name="const", bufs=1) as cpool,
            tc.tile_pool(name="win", bufs=1) as gpool,
            tc.tile_pool(name="hsb", bufs=3) as hsbpool,
            tc.tile_pool(name="osb", bufs=2) as osbpool,
            tc.tile_pool(name="hps", bufs=2, space="PSUM") as hpool,
            tc.tile_pool(name="ops", bufs=2, space="PSUM") as opool,
        ):  # PSUM: 2 tags x 2 bufs x 2 banks = all 8 banks
            # weight/bias loads on the ACT queue: tiny transfers that must
            # not delay the window stream on the Sync queue
            w1t_sb = cpool.tile([D, D], bf16, tag="w1t")
            nc.scalar.dma_start(out=w1t_sb[:], in_=w1t_t[:])
            w2t_sb = cpool.tile([D, D], bf16, tag="w2t")
            nc.scalar.dma_start(out=w2t_sb[:], in_=w2t_t[:])
            b1_sb = cpool.tile([P, 1], f32, tag="b1")
            nc.scalar.dma_start(out=b1_sb[:], in_=b1_t[:])
            b2_sb = cpool.tile([P, 1], f32, tag="b2")
            nc.scalar.dma_start(out=b2_sb[:], in_=b2_t[:])

            # every window DMA up-front: in-order Sync queue streams the
            # whole input without ever waiting behind an output DMA
            sts = []  # supertile -> (window tile, column offset)
            r0 = 0
            for ci, cb in enumerate(chunks):
                win = gpool.tile([P, cb * D], bf16, tag=f"win{ci}")
                nc.sync.dma_start(out=win[:], in_=tsl_t[:, r0 : r0 + cb * P])
                for s in range(cb // G):
                    sts.append((win, s * fw))
                r0 += cb * P

            def relu_bias(out_ap, in_ap, bias_sb, on_act: bool):
                if on_act:
                    nc.scalar.activation(out_ap, in_ap,
                                         mybir.ActivationFunctionType.Relu,
                                         bias=bias_sb[:, 0:1])
                else:
                    nc.vector.tensor_scalar(
                        out=out_ap, in0=in_ap, scalar1=bias_sb[:, 0:1],
                        scalar2=0.0, op0=mybir.AluOpType.add,
                        op1=mybir.AluOpType.max)

            # layer 1 is emitted supertile-granular (mm1 -> ACT relu1 into
            # a bf16 SBUF staging tile), layer 2 pair-granular (2x mm2
            # into a 2-bank PSUM tile -> DVE relu2 -> output DMA).  mm1
            # emission runs ahead of mm2 with a look-ahead that grows from
            # 2 supertiles (so the first outputs aren't gated on later
            # input chunks) to 6 (so the in-order PE queue never waits a
            # full relu1 latency in steady state).
            # pair-granular two-stage pipeline, software-pipelined one
            # pair deep: the PE queue holds mm1 of pair k+1 before mm2 of
            # pair k so the in-order PE never sits a full relu1 latency.
            # Output pairs are staged into quads (4 KB partition lines,
            # half the store DMAs).  Instruction (and thus semaphore)
            # count is kept low: the end-of-kernel semaphore-reset
            # cascade costs ~30 ns per allocated semaphore on the
            # critical teardown path.
            pairs = [sts[p0 : p0 + 2] for p0 in range(0, len(sts), 2)]
            n_pairs = len(pairs)
            hT = [None] * n_pairs

            def emit_mm1(k):
                h_ps = hpool.tile([P, pw], f32, tag="h")
                for j, (wt, off) in enumerate(pairs[k]):
                    nc.tensor.matmul(out=h_ps[:, j * fw : (j + 1) * fw],
                                     lhsT=w1t_sb[:],
                                     rhs=wt[:, off : off + fw],
                                     start=True, stop=True)
                w = len(pairs[k]) * fw
                hT[k] = hsbpool.tile([P, pw], bf16, tag="hT",
                                     name=f"hT{k}")
                relu_bias(hT[k][:, :w], h_ps[:, :w], b1_sb, on_act=True)

            emit_mm1(0)
            o_quad = None
            for k in range(n_pairs):
                w = len(pairs[k]) * fw
                if k + 1 < n_pairs:
                    emit_mm1(k + 1)

                o_ps = opool.tile([P, pw], f32, tag="o")
                for j in range(w // fw):
                    nc.tensor.matmul(out=o_ps[:, j * fw : (j + 1) * fw],
                                     lhsT=w2t_sb[:],
                                     rhs=hT[k][:, j * fw : (j + 1) * fw],
                                     start=True, stop=True)
                hT[k] = None
                if k % 2 == 0:
                    o_quad = osbpool.tile([P, 2 * pw], bf16, tag="o_sb")
                half = (k % 2) * pw
                relu_bias(o_quad[:, half : half + w], o_ps[:, :w], b2_sb,
                          on_act=False)
                if k % 2 == 1:
                    nc.sync.dma_start(
                        out=out_t[:, (k - 1) * pw : (k - 1) * pw + pw + w],
                        in_=o_quad[:, : pw + w])
                elif k == n_pairs - 1:
                    nc.sync.dma_start(
                        out=out_t[:, k * pw : k * pw + w],
                        in_=o_quad[:, :w])

    nc.compile()
    return nc


_CACHED_NC: dict = {}
LAST_RESULTS = None


def _get_nc(blocks: int):
    if blocks not in _CACHED_NC:
        _CACHED_NC[blocks] = build_nc(blocks)
    return _CACHED_NC[blocks]


def _run(in_maps):
    trace = os.environ.get("BASS_KERNEL_TRACE") == "1"
    if trace:
        try:  # tracing needs the NTFF hook; degrade silently without it
            import antenv.axon_hooks  # noqa: F401
        except ImportError:
            trace = False
    nc = _get_nc(in_maps[0]["tslice"].shape[1] // P)
    res = run_bass_kernel_spmd(nc, in_maps, core_ids=list(range(N_CORES)),
                               trace=trace)
    global LAST_RESULTS
    LAST_RESULTS = res
    return res


def kernel(nodes, c2e_weight, w1, b1, w2, b2):
    import ml_dtypes

    bf16 = ml_dtypes.bfloat16
    nodes = np.asarray(nodes)
    c2e_weight = np.asarray(c2e_weight, dtype=np.float32)
    w1 = np.asarray(w1, dtype=np.float32)
    b1 = np.asarray(b1, dtype=np.float32)
    w2 = np.asarray(w2, dtype=np.float32)
    b2 = np.asarray(b2, dtype=np.float32)

    vocab = c2e_weight.shape[0]
    assert vocab == VOCAB, vocab

    tableT = np.ascontiguousarray(c2e_weight.T.astype(bf16))  # [128, VOCAB]
    w1t = np.ascontiguousarray(w1.T.astype(bf16))
    w2t = np.ascontiguousarray(w2.T.astype(bf16))
    b1c = np.ascontiguousarray(b1.reshape(P, 1))
    b2c = np.ascontiguousarray(b2.reshape(P, 1))
    consts = {"w1t": w1t, "w2t": w2t, "b1c": b1c, "b2c": b2c}

    uniq = np.unique(nodes)
    cap = N_CORES * CAP_BLOCKS * P
    t2 = np.empty((vocab, D), dtype=np.float32)

    if len(uniq) <= cap:
        # compacted path: move/compute only referenced rows (padded with
        # duplicates of id 0, which all receive the same correct value)
        rows = CAP_BLOCKS * P
        ids = np.zeros(cap, dtype=np.int64)
        ids[: len(uniq)] = uniq
        packed = tableT[:, ids]  # [128, cap] bf16
        in_maps = []
        for i in range(N_CORES):
            in_maps.append({
                "tslice": np.ascontiguousarray(
                    packed[:, i * rows : (i + 1) * rows]),
                **consts,
            })
        res = _run(in_maps)
        for i in range(N_CORES):
            dense = res.results[i]["out"]  # [128, rows] bf16, (d, r)
            t2[ids[i * rows : (i + 1) * rows]] = \
                dense.T.astype(np.float32)
    else:
        # fallback: transform the whole table in fixed vocab ranges
        rows = FULL_BLOCKS * P
        starts, in_maps = [], []
        for i in range(N_CORES):
            start = min(i * RANGE, vocab - rows)
            starts.append(start)
            in_maps.append({
                "tslice": np.ascontiguousarray(
                    tableT[:, start : start + rows]),
                **consts,
            })
        res = _run(in_maps)
        for i in range(N_CORES):
            dense = res.results[i]["out"]
            lo = i * RANGE
            hi = min((i + 1) * RANGE, vocab)
            t2[lo:hi] = dense[:, lo - starts[i] : hi - starts[i]].T \
                .astype(np.float32)

    return t2[nodes]
